# revision 24
# baseline (speedup 1.0000x reference)
"""Linformer attention TRN2 Bass kernel.

Problem: nn_LinformerAttention (B=4, L=4096, D=1024, NH=16, DH=64, k=128).

Sharding: 8 cores = batch(4) x head-group(2). Core c handles batch c%4 and
heads (c//4)*8 .. +8, producing out[b, :, hg*512:(hg+1)*512]. Slices are
disjoint -> no collectives; host reassembles.

All matmul operands are bf16 (fp32 PSUM accumulation); bf16 streams 1
row/cycle on the PE vs 4 for fp32. rel-err budget is 2e-2, bf16 lands
~7e-3. fp8 was measured (host-sim) at ~6e-2 even with power-of-2
pre-scaling to dodge denormals — over budget, rejected.

Pass A uses the factored Linformer form: instead of materializing
K = x@Wk.T and V = x@Wv.T ([4096,512] each) and then reducing with E
(256 extra 128-row matmuls), compute per 4-head group g

    P_h.T[d, kk] = sum_l x[l, d] * E_h[kk, l]     (d on partitions)

directly: stationary = x l-tile [128 ll, 128 d-slice], moving = E.T
l-tile [128 ll, 512 (4 heads x kk)], accumulating over all 32 l-tiles
into 8 PSUM banks (one per d-slice). Same 512 big matmuls as the K/V
projections used to take, but the E-reduction comes for free. Then

    Kp_h.T[dh, kk] = Wk_h @ P_h.T   (8 small matmuls per head, + rank-1
                                     rowsum(E_h) x bk_h bias correction)
    Vp_h[kk, dh]   = P_h.T.T @ Wv_h.T  (same, + bias)

which also lands Kp.T directly in the layout the dot matmul needs (no
PE transposes, no DVE bias-adds, no K/V SBUF round-trip). Startup now
gates on 384 KiB (first x/E l-tiles) instead of 2 MiB of weights —
weights are only needed ~55us in and trickle in behind the paced
x/E stream.

pass B (per chunk of 512 l; x re-DMA'd in d-partition layout):
  - Q.T-chunk = Wq @ x.T + bq (scale 1/sqrt(dh) folded into Wq/bq on
    host), jt-pairs interleaved across two PSUM banks; bias-add+cast on
    the Scalar engine (ACT Identity with per-partition bias) so psQ bank
    recycling never waits on the DVE queue
  - per head (software-pipelined, DEPTH=3): dotT = KpT.T @ Q.T-chunk
    (the unused 64 partitions of kpT are zero so the matmul contracts
    the full 128-partition Q pair-tile); expT = exp(dotT) on Scalar;
    Xo_aug[l, lt, 65] = expT-tile.T @ [Vp|1] into one PSUM bank (col 64
    = softmax denominator); one reciprocal [128,4] + one broadcast-mult
  - out DMA per head-pair on the Sync queue

Scheduling notes (from perfetto traces): DMA descriptor generation
(DIRECT2D) costs ~0.7us per instruction on the issuing engine queue;
queues race ahead on gen, so transfer priority is controlled by
same-queue FIFO order and tile-pool pacing. Back-to-back matmuls
accumulating into the SAME PSUM bank stall the PE ~70ns each — the
P accumulation rotates 8 banks, pass B interleaves pairs.

Host prep (numpy, outside HW-timed region): x sent twice — natural
[lt, ll, d] tiling for pass A and d-partition tiling for pass B; W
pre-transposed+pre-permuted to [pi, dc, j] (+1/8 scale on Wq); E
pre-transposed to [lt, ll, g, h4*kk] plus rowsums; all bf16 except
bqT kept fp32.
"""

import sys

sys.path.insert(0, "/opt/trn_rl_repo")

import math
from contextlib import ExitStack

import numpy as np
import ml_dtypes

import json

import concourse.bass as bass
import concourse.bass2jax as bass2jax
import concourse.mybir as mybir
import concourse.tile as tile
from concourse.bass_utils import compile_bir_kernel as _orig_compile_bir_kernel
from concourse.bass_utils import run_bass_kernel_spmd


def _split_multiwaits(bir_json_bytes):
    """This container's walrus encodes at most ONE sync wait per engine
    instruction ("Too many sync wait commands" otherwise), while Tile emits
    multi-wait instructions. Hoist extra waits onto single-wait
    EventSemaphore carrier instructions placed just before, on the same
    engine queue — semantically identical stalling."""
    bj = json.loads(bir_json_bytes)
    for fn in bj["functions"]:
        for blk in fn["blocks"]:
            out = []
            for inst in blk["instructions"]:
                si = inst.get("sync_info")
                waits = (si or {}).get("on_wait") or []
                if si and len(waits) > 1:
                    for wi, w in enumerate(waits[:-1]):
                        out.append(
                            {
                                "debug": inst.get("debug", 0),
                                "engine": inst.get("engine"),
                                "ins": [],
                                "outs": [],
                                "name": inst["name"] + "-w%d" % wi,
                                "opcode": "EventSemaphore",
                                "sync_info": {"on_update": [], "on_wait": [w]},
                            }
                        )
                    si["on_wait"] = [waits[-1]]
                out.append(inst)
            blk["instructions"] = out
    return json.dumps(bj).encode()


def _patched_compile_bir_kernel(bir_json, tmpdir, neff_name="file.neff"):
    return _orig_compile_bir_kernel(_split_multiwaits(bir_json), tmpdir, neff_name)


bass2jax.compile_bir_kernel = _patched_compile_bir_kernel

B, L, D = 4, 4096, 1024
NH, DH, KK = 16, 64, 128
NCORES = 8
HGS = 2  # head groups (core-level sharding)
H = NH // HGS  # 8 local heads per core
J = H * DH  # 512 output columns per core
P = 128
LCH = 512  # l-chunk (pass B)
NLC = L // LCH  # 8
NLT = L // P  # 32 l-tiles (pass A)
DC = D // P  # 8 contraction subtiles
JT = J // P  # 4
LT4 = LCH // P  # 4 l-tiles per chunk
G4 = 2  # pass-A head groups of 4
KK4 = 4 * KK  # 512
F32 = mybir.dt.float32
BF16 = mybir.dt.bfloat16

TRACE = False  # test.py sets True to collect a profile
LAST_RESULTS = None  # BassKernelResults of the last kernel() call

_PROGRAM = {}


def _build_program(has_bias):
    nc = bass.Bass()
    # pass-A x: natural row-major tiling [lt, ll, d] (one DMA per l-tile,
    # 2 KiB/partition contiguous lines)
    xL = nc.declare_dram_parameter("xL", [NLT, P, D], BF16, isOutput=False)
    # pass-B x: [lc, pi, dc, lt, ll] so the Q moving operand is contiguous
    xT = nc.declare_dram_parameter("xT", [NLC, P, DC, LT4, P], BF16, isOutput=False)
    # weights pre-permuted on host to [pi, dc, j]
    wqP = nc.declare_dram_parameter("wqP", [P, DC, J], BF16, isOutput=False)
    wkP = nc.declare_dram_parameter("wkP", [P, DC, J], BF16, isOutput=False)
    wvP = nc.declare_dram_parameter("wvP", [P, DC, J], BF16, isOutput=False)
    bqT = nc.declare_dram_parameter("bqT", [P, JT], F32, isOutput=False)
    # E.T tiles: [lt, ll, g, 4-head x kk]
    eL = nc.declare_dram_parameter("eL", [NLT, P, G4, KK4], BF16, isOutput=False)
    # rank-1 bias-correction operands: rowsum(E_h) over l, and bk/bv per head
    erowsT = nc.declare_dram_parameter("erowsT", [1, H, KK], BF16, isOutput=False)
    bkv = nc.declare_dram_parameter("bkv", [1, 2, H, DH], BF16, isOutput=False)
    # bf16 output (host upcasts): halves the out-DMA bytes — each DMA
    # instruction runs on ~one engine, so size directly sets drain latency
    out = nc.declare_dram_parameter("out", [L, J], BF16, isOutput=True)

    mult = mybir.AluOpType.mult

    with tile.TileContext(nc) as tc:
        with ExitStack() as ctx:
            const = ctx.enter_context(tc.tile_pool(name="const", bufs=1))
            epool = ctx.enter_context(tc.tile_pool(name="e", bufs=8))
            ptpool = ctx.enter_context(tc.tile_pool(name="pt", bufs=2))
            xpool = ctx.enter_context(tc.tile_pool(name="x", bufs=2))
            qtpool = ctx.enter_context(tc.tile_pool(name="qt", bufs=2))
            exppool = ctx.enter_context(tc.tile_pool(name="ex", bufs=4))
            outpool = ctx.enter_context(tc.tile_pool(name="ot", bufs=3))
            recpool = ctx.enter_context(tc.tile_pool(name="rc", bufs=4))
            psA = ctx.enter_context(tc.tile_pool(name="psA", bufs=4, space="PSUM"))
            psB = ctx.enter_context(tc.tile_pool(name="psB", bufs=2, space="PSUM"))
            psXp = ctx.enter_context(tc.tile_pool(name="psX", bufs=2, space="PSUM"))

            # ---- constants resident in SBUF (loads issued mid-pass-A; see below)
            wq_sb = const.tile([P, DC, J], BF16, tag="wq")
            wk_sb = const.tile([P, DC, J], BF16, tag="wk")
            wv_sb = const.tile([P, DC, J], BF16, tag="wv")
            bqT_sb = const.tile([P, JT], F32, tag="bqT")
            erowsT_sb = const.tile([1, H, KK], BF16, tag="erowsT")
            bkv_sb = const.tile([1, 2, H, DH], BF16, tag="bkv")

            # x resident for all of pass A (both head groups read it)
            xres = const.tile([P, NLT, D], BF16, tag="xres")
            gate = const.tile([1, 1], BF16, tag="gate")

            # per-head Kp.T for the dot matmul: head h occupies partitions
            # (h%2)*64..+64, the other 64 partitions are ZERO so the matmul can
            # contract all 128 partitions of the shared Q pair-tile.
            kpT = [const.tile([P, KK], BF16, tag=f"kpT{h}", name=f"kpT{h}") for h in range(H)]
            vpa = [const.tile([P, DH + 1], BF16, tag=f"vpa{h}", name=f"vpa{h}") for h in range(H)]
            for h in range(H):
                b0z = ((h + 1) % 2) * DH  # the half that must stay zero
                nc.vector.memset(kpT[h][b0z : b0z + DH, :], 0.0)
                nc.vector.memset(vpa[h][:, DH : DH + 1], 1.0)

            outr = out[:, :].rearrange("(lo li) j -> li lo j", li=P)

            # ---- pass A: P.T accumulation + Kp.T/Vp extraction, per 4-head group
            for g in range(G4):
                psPs = []
                for dt in range(DC):
                    # one PSUM bank per d-slice: 4 from psA, 2 from psB, 2 from
                    # psXp (tag must match each pool's existing tag — pools
                    # size their buffers per tag)
                    pool, tg = ((psA, "big"), (psB, "big"), (psXp, "x4"))[
                        0 if dt < 4 else (1 if dt < 6 else 2)
                    ]
                    psPs.append(pool.tile([P, KK4], F32, tag=tg, name=f"psP{g}_{dt}"))
                for lt in range(NLT):
                    if g == 0:
                        if lt == 0:
                            # first tiles split across several DMA instructions:
                            # each instruction lands on ~one DMA engine
                            # (~25-35 GB/s), so splitting parallelizes engines
                            # and the first matmul gates on ~64 KiB
                            for q in range(4):
                                nc.gpsimd.dma_start(
                                    xres[:, 0, q * 256 : (q + 1) * 256],
                                    xL[0, :, q * 256 : (q + 1) * 256],
                                )
                        elif lt in (1, 2):
                            for q in range(2):
                                nc.gpsimd.dma_start(
                                    xres[:, lt, q * 512 : (q + 1) * 512],
                                    xL[lt, :, q * 512 : (q + 1) * 512],
                                )
                        elif lt % 2 == 1:
                            # odd tiles on the Scalar queue: two queues double
                            # both the early engine parallelism and the
                            # outstanding-DMA ring runway
                            nc.scalar.dma_start(xres[:, lt, :], xL[lt])
                        else:
                            nc.gpsimd.dma_start(xres[:, lt, :], xL[lt])
                    els = epool.tile([P, KK4], BF16, tag="el")
                    if g == 0 and lt < 2:
                        nc.sync.dma_start(els[:, 0:256], eL[lt, :, g, 0:256])
                        nc.sync.dma_start(els[:, 256:512], eL[lt, :, g, 256:512])
                    else:
                        nc.sync.dma_start(els[:], eL[lt, :, g])
                    if g == 0 and lt == 16:
                        if True:
                            # weights/constants first needed at ~55us (Kp/Vp)
                            # and in pass B. The Scalar queue is idle in pass A
                            # but races ahead, so gate it on a mid-pass-A
                            # E-tile: the 3 MiB burst then starts ~t+28us, on
                            # its own queue's rings, never touching the
                            # startup-critical x/E streams.
                            nc.scalar.activation(
                                gate[:], els[0:1, 0:1],
                                mybir.ActivationFunctionType.Copy,
                            )
                            nc.scalar.dma_start(wk_sb[:], wkP[:, :, :])
                            nc.scalar.dma_start(wv_sb[:], wvP[:, :, :])
                            nc.scalar.dma_start(wq_sb[:], wqP[:, :, :])
                            nc.scalar.dma_start(bqT_sb[:], bqT[:, :])
                            nc.scalar.dma_start(erowsT_sb[:], erowsT[0:1])
                            nc.scalar.dma_start(bkv_sb[:], bkv[0:1])
                    for dt in range(DC):
                        nc.tensor.matmul(
                            psPs[dt][:], xres[:, lt, dt * P : (dt + 1) * P], els[:],
                            start=(lt == 0), stop=(lt == NLT - 1),
                        )
                pt = ptpool.tile([P, DC, KK4], BF16, tag="pt")
                for dt in range(DC):
                    nc.any.tensor_copy(pt[:, dt, :], psPs[dt][:])
                for tp in range(2):  # head pairs within the group
                    psKpT = psA.tile([P, KK], F32, tag="big", name=f"psKpT{g}_{tp}")
                    psVp = psA.tile([P, 2, DH], F32, tag="big", name=f"psVp{g}_{tp}")
                    for par in range(2):
                        h = g * 4 + tp * 2 + par  # local head index
                        hh = tp * 2 + par  # head within group
                        b0 = par * DH
                        for dt in range(DC):
                            nc.tensor.matmul(
                                psKpT[b0 : b0 + DH, :],
                                wk_sb[:, dt, h * DH : (h + 1) * DH],
                                pt[:, dt, hh * KK : (hh + 1) * KK],
                                start=(dt == 0),
                                stop=(not has_bias and dt == DC - 1),
                            )
                        if has_bias:
                            nc.tensor.matmul(
                                psKpT[b0 : b0 + DH, :],
                                bkv_sb[0:1, 0, h, :], erowsT_sb[0:1, h, :],
                                start=False, stop=True,
                            )
                        for dt in range(DC):
                            nc.tensor.matmul(
                                psVp[:, par, :],
                                pt[:, dt, hh * KK : (hh + 1) * KK],
                                wv_sb[:, dt, h * DH : (h + 1) * DH],
                                start=(dt == 0),
                                stop=(not has_bias and dt == DC - 1),
                            )
                        if has_bias:
                            nc.tensor.matmul(
                                psVp[:, par, :],
                                erowsT_sb[0:1, h, :], bkv_sb[0:1, 1, h, :],
                                start=False, stop=True,
                            )
                    for par in range(2):
                        h = g * 4 + tp * 2 + par
                        b0 = par * DH
                        nc.any.tensor_copy(kpT[h][b0 : b0 + DH, :], psKpT[b0 : b0 + DH, :])
                        nc.any.tensor_copy(vpa[h][:, 0:DH], psVp[:, par, :])

            # ---- pass B: Q projection fused with attention, per chunk
            DEPTH = 3  # psD/exp issued this many heads ahead of psX
            for lc in range(NLC):
                x_sb = xpool.tile([P, DC, LT4, P], BF16, tag="x")
                nc.gpsimd.dma_start(x_sb[:, 0:4], xT[lc, :, 0:4])
                nc.gpsimd.dma_start(x_sb[:, 4:DC], xT[lc, :, 4:DC])
                qt = qtpool.tile([P, JT, LCH], BF16, tag="qt")
                # jt-pairs interleaved across the two psB banks (same-bank
                # accumulation bubble). psQ draws from psB so psD keeps all 4
                # psA banks: dot_h then only waits on exp_{h-4}.
                for jt0 in (0, 2):
                    psQs = [psB.tile([P, LCH], F32, tag="big", name=f"psQ{i}") for i in range(2)]
                    for dc in range(DC):
                        for i in range(2):
                            jt = jt0 + i
                            nc.tensor.matmul(
                                psQs[i][:], wq_sb[:, dc, jt * P : (jt + 1) * P],
                                x_sb[:, dc],
                                start=(dc == 0), stop=(dc == DC - 1),
                            )
                    for i in range(2):
                        jt = jt0 + i
                        # bias-add on the Scalar engine (ACT: out = f(in*1+bias))
                        nc.scalar.activation(
                            qt[:, jt, :], psQs[i][:],
                            mybir.ActivationFunctionType.Identity,
                            bias=bqT_sb[:, jt : jt + 1],
                        )
                ot = outpool.tile([P, LT4, J], BF16, tag="ot")
                exs = [None] * H
                for hh in range(H + DEPTH):
                    if hh < H:
                        h = hh
                        psD = psA.tile([P, LCH], F32, tag="big")
                        nc.tensor.matmul(
                            psD[:], kpT[h][:],
                            qt[:, h // 2, :],
                            start=True, stop=True,
                        )
                        ex = exppool.tile([P, LCH], BF16, tag="ex")
                        nc.scalar.activation(
                            ex[:], psD[:], mybir.ActivationFunctionType.Exp
                        )
                        exs[h] = ex
                    if hh >= DEPTH:
                        h = hh - DEPTH
                        ex = exs[h]
                        psX = psXp.tile([P, LT4, DH + 1], F32, tag="x4")
                        for lt in range(LT4):
                            nc.tensor.matmul(
                                psX[:, lt, :], ex[:, lt * P : (lt + 1) * P],
                                vpa[h][:],
                                start=True, stop=True,
                            )
                        rc = recpool.tile([P, LT4, 1], F32, tag="rc")
                        nc.vector.reciprocal(rc[:], psX[:, :, DH : DH + 1])
                        nc.vector.tensor_tensor(
                            ot[:, :, h * DH : (h + 1) * DH],
                            psX[:, :, 0:DH],
                            rc[:].to_broadcast([P, LT4, DH]),
                            mult,
                        )
                        j0 = h * DH
                        nc.sync.dma_start(
                            outr[:, lc * LT4 : (lc + 1) * LT4, j0 : j0 + DH],
                            ot[:, :, j0 : j0 + DH],
                        )

    return nc


def _get_program(has_bias):
    if has_bias not in _PROGRAM:
        _PROGRAM[has_bias] = _build_program(has_bias)
    return _PROGRAM[has_bias]


def kernel(x, Wq, bq, Wk, bk, Wv, bv, E):
    global LAST_RESULTS
    x = np.asarray(x, dtype=np.float32)
    Wq = np.asarray(Wq, dtype=np.float32)
    bq = np.asarray(bq, dtype=np.float32)
    Wk = np.asarray(Wk, dtype=np.float32)
    bk = np.asarray(bk, dtype=np.float32)
    Wv = np.asarray(Wv, dtype=np.float32)
    bv = np.asarray(bv, dtype=np.float32)
    E = np.asarray(E, dtype=np.float32)

    BF = ml_dtypes.bfloat16
    scale = 1.0 / math.sqrt(DH)
    # pass-A x: [lt, ll, d] — literally x[b] reshaped
    xLs = [np.ascontiguousarray(x[b].reshape(NLT, P, D).astype(BF)) for b in range(B)]
    # pass-B x: [d, l] -> [lc, pi, dc, lt, ll]
    xTs = [
        np.ascontiguousarray(
            x[b].T.reshape(DC, P, NLC, LT4, P).transpose(2, 1, 0, 3, 4).astype(BF)
        )
        for b in range(B)
    ]
    in_maps = []
    for core in range(NCORES):
        b = core % B
        hg = core // B
        js = slice(hg * J, (hg + 1) * J)
        hs = slice(hg * H, (hg + 1) * H)
        # W.T [D, J] -> [pi, dc, j] so per-partition lines are contiguous
        wqPs = np.ascontiguousarray(
            (Wq[js, :] * scale).T.reshape(DC, P, J).transpose(1, 0, 2).astype(BF)
        )
        wkPs = np.ascontiguousarray(
            Wk[js, :].T.reshape(DC, P, J).transpose(1, 0, 2).astype(BF)
        )
        wvPs = np.ascontiguousarray(
            Wv[js, :].T.reshape(DC, P, J).transpose(1, 0, 2).astype(BF)
        )
        bqTs = np.ascontiguousarray((bq[js] * scale).reshape(JT, P).T)
        E_s = E[hs]  # [H, KK, L]
        # [g, h4, kk, lt, ll] -> [lt, ll, g, h4*kk]
        eLs = np.ascontiguousarray(
            E_s.reshape(G4, 4, KK, NLT, P).transpose(3, 4, 0, 1, 2)
            .reshape(NLT, P, G4, KK4).astype(BF)
        )
        erowsTs = np.ascontiguousarray(E_s.sum(-1).reshape(1, H, KK).astype(BF))
        bkvs = np.ascontiguousarray(
            np.stack([bk[js].reshape(H, DH), bv[js].reshape(H, DH)])
            .reshape(1, 2, H, DH).astype(BF)
        )
        in_maps.append(
            {
                "xL": xLs[b],
                "xT": xTs[b],
                "wqP": wqPs,
                "wkP": wkPs,
                "wvP": wvPs,
                "bqT": bqTs,
                "eL": eLs,
                "erowsT": erowsTs,
                "bkv": bkvs,
            }
        )

    has_bias = bool(np.any(bk) or np.any(bv))
    nc = _get_program(has_bias)
    res = run_bass_kernel_spmd(nc, in_maps, list(range(NCORES)), trace=TRACE)
    LAST_RESULTS = res

    outp = np.empty((B, L, D), dtype=np.float32)
    for core in range(NCORES):
        b = core % B
        hg = core // B
        outp[b, :, hg * J : (hg + 1) * J] = res.results[core]["out"].astype(np.float32)
    return outp


# revision 25
# speedup vs baseline: 1.1970x; 1.1970x over previous
"""Linformer attention TRN2 Bass kernel.

Problem: nn_LinformerAttention (B=4, L=4096, D=1024, NH=16, DH=64, k=128).

Sharding: 8 cores = batch(4) x head-group(2). Core c handles batch c%4 and
heads (c//4)*8 .. +8, producing out[b, :, hg*512:(hg+1)*512]. Slices are
disjoint -> no collectives; host reassembles.

All matmul operands are bf16 (fp32 PSUM accumulation); bf16 streams 1
row/cycle on the PE vs 4 for fp32. rel-err budget is 2e-2, bf16 lands
~7e-3. fp8 was measured (host-sim) at ~6e-2 even with power-of-2
pre-scaling to dodge denormals — over budget, rejected.

Pass A uses the factored Linformer form: instead of materializing
K = x@Wk.T and V = x@Wv.T ([4096,512] each) and then reducing with E
(256 extra 128-row matmuls), compute per 4-head group g

    P_h.T[d, kk] = sum_l x[l, d] * E_h[kk, l]     (d on partitions)

directly: stationary = x l-tile [128 ll, 128 d-slice], moving = E.T
l-tile [128 ll, 512 (4 heads x kk)], accumulating over all 32 l-tiles
into 8 PSUM banks (one per d-slice). Same 512 big matmuls as the K/V
projections used to take, but the E-reduction comes for free. Then

    Kp_h.T[dh, kk] = Wk_h @ P_h.T   (8 small matmuls per head, + rank-1
                                     rowsum(E_h) x bk_h bias correction)
    Vp_h[kk, dh]   = P_h.T.T @ Wv_h.T  (same, + bias)

which also lands Kp.T directly in the layout the dot matmul needs (no
PE transposes, no DVE bias-adds, no K/V SBUF round-trip). Startup now
gates on 384 KiB (first x/E l-tiles) instead of 2 MiB of weights —
weights are only needed ~55us in and trickle in behind the paced
x/E stream.

pass B (per chunk of 512 l; x re-DMA'd in d-partition layout):
  - Q.T-chunk = Wq @ x.T + bq (scale 1/sqrt(dh) folded into Wq/bq on
    host), jt-pairs interleaved across two PSUM banks; bias-add+cast on
    the Scalar engine (ACT Identity with per-partition bias) so psQ bank
    recycling never waits on the DVE queue
  - per head (software-pipelined, DEPTH=3): dotT = KpT.T @ Q.T-chunk
    (the unused 64 partitions of kpT are zero so the matmul contracts
    the full 128-partition Q pair-tile); expT = exp(dotT) on Scalar;
    Xo_aug[l, lt, 65] = expT-tile.T @ [Vp|1] into one PSUM bank (col 64
    = softmax denominator); one reciprocal [128,4] + one broadcast-mult
  - out DMA per head-pair on the Sync queue

Scheduling notes (from perfetto traces): DMA descriptor generation
(DIRECT2D) costs ~0.7us per instruction on the issuing engine queue;
queues race ahead on gen, so transfer priority is controlled by
same-queue FIFO order and tile-pool pacing. Back-to-back matmuls
accumulating into the SAME PSUM bank stall the PE ~70ns each — the
P accumulation rotates 8 banks, pass B interleaves pairs.

Host prep (numpy, outside HW-timed region): x sent twice — natural
[lt, ll, d] tiling for pass A and d-partition tiling for pass B; W
pre-transposed+pre-permuted to [pi, dc, j] (+1/8 scale on Wq); E
pre-transposed to [lt, ll, g, h4*kk] plus rowsums; all bf16 except
bqT kept fp32.
"""

import sys

sys.path.insert(0, "/opt/trn_rl_repo")

import math
from contextlib import ExitStack

import numpy as np
import ml_dtypes

import json

import concourse.bass as bass
import concourse.bass2jax as bass2jax
import concourse.mybir as mybir
import concourse.tile as tile
from concourse.bass_utils import compile_bir_kernel as _orig_compile_bir_kernel
from concourse.bass_utils import run_bass_kernel_spmd


def _split_multiwaits(bir_json_bytes):
    """This container's walrus encodes at most ONE sync wait per engine
    instruction ("Too many sync wait commands" otherwise), while Tile emits
    multi-wait instructions. Hoist extra waits onto single-wait
    EventSemaphore carrier instructions placed just before, on the same
    engine queue — semantically identical stalling."""
    bj = json.loads(bir_json_bytes)
    for fn in bj["functions"]:
        for blk in fn["blocks"]:
            out = []
            for inst in blk["instructions"]:
                si = inst.get("sync_info")
                waits = (si or {}).get("on_wait") or []
                if si and len(waits) > 1:
                    for wi, w in enumerate(waits[:-1]):
                        out.append(
                            {
                                "debug": inst.get("debug", 0),
                                "engine": inst.get("engine"),
                                "ins": [],
                                "outs": [],
                                "name": inst["name"] + "-w%d" % wi,
                                "opcode": "EventSemaphore",
                                "sync_info": {"on_update": [], "on_wait": [w]},
                            }
                        )
                    si["on_wait"] = [waits[-1]]
                out.append(inst)
            blk["instructions"] = out
    return json.dumps(bj).encode()


def _patched_compile_bir_kernel(bir_json, tmpdir, neff_name="file.neff"):
    return _orig_compile_bir_kernel(_split_multiwaits(bir_json), tmpdir, neff_name)


bass2jax.compile_bir_kernel = _patched_compile_bir_kernel

B, L, D = 4, 4096, 1024
NH, DH, KK = 16, 64, 128
NCORES = 8
HGS = 2  # head groups (core-level sharding)
H = NH // HGS  # 8 local heads per core
J = H * DH  # 512 output columns per core
P = 128
LCH = 512  # l-chunk (pass B)
NLC = L // LCH  # 8
NLT = L // P  # 32 l-tiles (pass A)
DC = D // P  # 8 contraction subtiles
JT = J // P  # 4
LT4 = LCH // P  # 4 l-tiles per chunk
G4 = 2  # pass-A head groups of 4
KK4 = 4 * KK  # 512
F32 = mybir.dt.float32
BF16 = mybir.dt.bfloat16

TRACE = False  # test.py sets True to collect a profile
LAST_RESULTS = None  # BassKernelResults of the last kernel() call

_PROGRAM = {}


def _build_program(has_bias):
    nc = bass.Bass()
    # pass-A x: natural row-major tiling [lt, ll, d] (one DMA per l-tile,
    # 2 KiB/partition contiguous lines)
    xL = nc.declare_dram_parameter("xL", [NLT, P, D], BF16, isOutput=False)
    # pass-B x: [lc, pi, dc, lt, ll] so the Q moving operand is contiguous
    xT = nc.declare_dram_parameter("xT", [NLC, P, DC, LT4, P], BF16, isOutput=False)
    # weights pre-permuted on host to [pi, dc, j]
    wqP = nc.declare_dram_parameter("wqP", [P, DC, J], BF16, isOutput=False)
    wkP = nc.declare_dram_parameter("wkP", [P, DC, J], BF16, isOutput=False)
    wvP = nc.declare_dram_parameter("wvP", [P, DC, J], BF16, isOutput=False)
    bqT = nc.declare_dram_parameter("bqT", [P, JT], F32, isOutput=False)
    # E.T tiles: [lt, ll, g, 4-head x kk]
    eL = nc.declare_dram_parameter("eL", [NLT, P, G4, KK4], BF16, isOutput=False)
    # rank-1 bias-correction operands: rowsum(E_h) over l, and bk/bv per head
    erowsT = nc.declare_dram_parameter("erowsT", [1, H, KK], BF16, isOutput=False)
    bkv = nc.declare_dram_parameter("bkv", [1, 2, H, DH], BF16, isOutput=False)
    # bf16 output (host upcasts): halves the out-DMA bytes — each DMA
    # instruction runs on ~one engine, so size directly sets drain latency
    out = nc.declare_dram_parameter("out", [L, J], BF16, isOutput=True)

    mult = mybir.AluOpType.mult

    with tile.TileContext(nc) as tc:
        with ExitStack() as ctx:
            const = ctx.enter_context(tc.tile_pool(name="const", bufs=1))
            epool = ctx.enter_context(tc.tile_pool(name="e", bufs=8))
            ptpool = ctx.enter_context(tc.tile_pool(name="pt", bufs=2))
            xpool = ctx.enter_context(tc.tile_pool(name="x", bufs=2))
            qtpool = ctx.enter_context(tc.tile_pool(name="qt", bufs=2))
            exppool = ctx.enter_context(tc.tile_pool(name="ex", bufs=4))
            outpool = ctx.enter_context(tc.tile_pool(name="ot", bufs=3))
            recpool = ctx.enter_context(tc.tile_pool(name="rc", bufs=4))
            psA = ctx.enter_context(tc.tile_pool(name="psA", bufs=4, space="PSUM"))
            psB = ctx.enter_context(tc.tile_pool(name="psB", bufs=2, space="PSUM"))
            psXp = ctx.enter_context(tc.tile_pool(name="psX", bufs=2, space="PSUM"))

            # ---- constants resident in SBUF (loads issued mid-pass-A; see below)
            wq_sb = const.tile([P, DC, J], BF16, tag="wq")
            wk_sb = const.tile([P, DC, J], BF16, tag="wk")
            wv_sb = const.tile([P, DC, J], BF16, tag="wv")
            bqT_sb = const.tile([P, JT], F32, tag="bqT")
            erowsT_sb = const.tile([1, H, KK], BF16, tag="erowsT")
            bkv_sb = const.tile([1, 2, H, DH], BF16, tag="bkv")

            # x resident for all of pass A (both head groups read it)
            xres = const.tile([P, NLT, D], BF16, tag="xres")
            gate = const.tile([1, 1], BF16, tag="gate")

            # per-head Kp.T for the dot matmul: head h occupies partitions
            # (h%2)*64..+64, the other 64 partitions are ZERO so the matmul can
            # contract all 128 partitions of the shared Q pair-tile.
            kpT = [const.tile([P, KK], BF16, tag=f"kpT{h}", name=f"kpT{h}") for h in range(H)]
            vpa = [const.tile([P, DH + 1], BF16, tag=f"vpa{h}", name=f"vpa{h}") for h in range(H)]
            for h in range(H):
                b0z = ((h + 1) % 2) * DH  # the half that must stay zero
                nc.vector.memset(kpT[h][b0z : b0z + DH, :], 0.0)
                nc.vector.memset(vpa[h][:, DH : DH + 1], 1.0)

            outr = out[:, :].rearrange("(lo li) j -> li lo j", li=P)

            # ---- pass A: P.T accumulation + Kp.T/Vp extraction, per 4-head group
            for g in range(G4):
                psPs = []
                for dt in range(DC):
                    # one PSUM bank per d-slice: 4 from psA, 2 from psB, 2 from
                    # psXp (tag must match each pool's existing tag — pools
                    # size their buffers per tag)
                    pool, tg = ((psA, "big"), (psB, "big"), (psXp, "x4"))[
                        0 if dt < 4 else (1 if dt < 6 else 2)
                    ]
                    psPs.append(pool.tile([P, KK4], F32, tag=tg, name=f"psP{g}_{dt}"))
                for lt in range(NLT):
                    if g == 0:
                        if lt == 0:
                            # first tiles split across several DMA instructions:
                            # each instruction lands on ~one DMA engine
                            # (~25-35 GB/s), so splitting parallelizes engines
                            # and the first matmul gates on ~64 KiB
                            for q in range(4):
                                nc.gpsimd.dma_start(
                                    xres[:, 0, q * 256 : (q + 1) * 256],
                                    xL[0, :, q * 256 : (q + 1) * 256],
                                )
                        elif lt in (1, 2):
                            for q in range(2):
                                nc.gpsimd.dma_start(
                                    xres[:, lt, q * 512 : (q + 1) * 512],
                                    xL[lt, :, q * 512 : (q + 1) * 512],
                                )
                        else:
                            nc.gpsimd.dma_start(xres[:, lt, :], xL[lt])
                    els = epool.tile([P, KK4], BF16, tag="el")
                    if g == 0 and lt < 2:
                        nc.sync.dma_start(els[:, 0:256], eL[lt, :, g, 0:256])
                        nc.sync.dma_start(els[:, 256:512], eL[lt, :, g, 256:512])
                    else:
                        nc.sync.dma_start(els[:], eL[lt, :, g])
                    if g == 0 and lt == 16:
                        if True:
                            # weights/constants first needed at ~55us (Kp/Vp)
                            # and in pass B. The Scalar queue is idle in pass A
                            # but races ahead, so gate it on a mid-pass-A
                            # E-tile: the 3 MiB burst then starts ~t+28us, on
                            # its own queue's rings, never touching the
                            # startup-critical x/E streams.
                            nc.scalar.activation(
                                gate[:], els[0:1, 0:1],
                                mybir.ActivationFunctionType.Copy,
                            )
                            nc.scalar.dma_start(wk_sb[:], wkP[:, :, :])
                            nc.scalar.dma_start(wv_sb[:], wvP[:, :, :])
                            nc.scalar.dma_start(wq_sb[:], wqP[:, :, :])
                            nc.scalar.dma_start(bqT_sb[:], bqT[:, :])
                            nc.scalar.dma_start(erowsT_sb[:], erowsT[0:1])
                            nc.scalar.dma_start(bkv_sb[:], bkv[0:1])
                    for dt in range(DC):
                        nc.tensor.matmul(
                            psPs[dt][:], xres[:, lt, dt * P : (dt + 1) * P], els[:],
                            start=(lt == 0), stop=(lt == NLT - 1),
                        )
                pt = ptpool.tile([P, DC, KK4], BF16, tag="pt")
                for dt in range(DC):
                    nc.any.tensor_copy(pt[:, dt, :], psPs[dt][:])
                for tp in range(2):  # head pairs within the group
                    psKpT = psA.tile([P, KK], F32, tag="big", name=f"psKpT{g}_{tp}")
                    psVp = psA.tile([P, 2, DH], F32, tag="big", name=f"psVp{g}_{tp}")
                    for par in range(2):
                        h = g * 4 + tp * 2 + par  # local head index
                        hh = tp * 2 + par  # head within group
                        b0 = par * DH
                        for dt in range(DC):
                            nc.tensor.matmul(
                                psKpT[b0 : b0 + DH, :],
                                wk_sb[:, dt, h * DH : (h + 1) * DH],
                                pt[:, dt, hh * KK : (hh + 1) * KK],
                                start=(dt == 0),
                                stop=(not has_bias and dt == DC - 1),
                            )
                        if has_bias:
                            nc.tensor.matmul(
                                psKpT[b0 : b0 + DH, :],
                                bkv_sb[0:1, 0, h, :], erowsT_sb[0:1, h, :],
                                start=False, stop=True,
                            )
                        for dt in range(DC):
                            nc.tensor.matmul(
                                psVp[:, par, :],
                                pt[:, dt, hh * KK : (hh + 1) * KK],
                                wv_sb[:, dt, h * DH : (h + 1) * DH],
                                start=(dt == 0),
                                stop=(not has_bias and dt == DC - 1),
                            )
                        if has_bias:
                            nc.tensor.matmul(
                                psVp[:, par, :],
                                erowsT_sb[0:1, h, :], bkv_sb[0:1, 1, h, :],
                                start=False, stop=True,
                            )
                    for par in range(2):
                        h = g * 4 + tp * 2 + par
                        b0 = par * DH
                        nc.any.tensor_copy(kpT[h][b0 : b0 + DH, :], psKpT[b0 : b0 + DH, :])
                        nc.any.tensor_copy(vpa[h][:, 0:DH], psVp[:, par, :])

            # ---- pass B: Q projection fused with attention, per chunk
            DEPTH = 3  # psD/exp issued this many heads ahead of psX
            for lc in range(NLC):
                x_sb = xpool.tile([P, DC, LT4, P], BF16, tag="x")
                nc.gpsimd.dma_start(x_sb[:, 0:4], xT[lc, :, 0:4])
                nc.gpsimd.dma_start(x_sb[:, 4:DC], xT[lc, :, 4:DC])
                qt = qtpool.tile([P, JT, LCH], BF16, tag="qt")
                # jt-pairs interleaved across the two psB banks (same-bank
                # accumulation bubble). psQ draws from psB so psD keeps all 4
                # psA banks: dot_h then only waits on exp_{h-4}.
                for jt0 in (0, 2):
                    psQs = [psB.tile([P, LCH], F32, tag="big", name=f"psQ{i}") for i in range(2)]
                    for dc in range(DC):
                        for i in range(2):
                            jt = jt0 + i
                            nc.tensor.matmul(
                                psQs[i][:], wq_sb[:, dc, jt * P : (jt + 1) * P],
                                x_sb[:, dc],
                                start=(dc == 0), stop=(dc == DC - 1),
                            )
                    for i in range(2):
                        jt = jt0 + i
                        # bias-add on the Scalar engine (ACT: out = f(in*1+bias))
                        nc.scalar.activation(
                            qt[:, jt, :], psQs[i][:],
                            mybir.ActivationFunctionType.Identity,
                            bias=bqT_sb[:, jt : jt + 1],
                        )
                ot = outpool.tile([P, LT4, J], BF16, tag="ot")
                exs = [None] * H
                for hh in range(H + DEPTH):
                    if hh < H:
                        h = hh
                        psD = psA.tile([P, LCH], F32, tag="big")
                        nc.tensor.matmul(
                            psD[:], kpT[h][:],
                            qt[:, h // 2, :],
                            start=True, stop=True,
                        )
                        ex = exppool.tile([P, LCH], BF16, tag="ex")
                        nc.scalar.activation(
                            ex[:], psD[:], mybir.ActivationFunctionType.Exp
                        )
                        exs[h] = ex
                    if hh >= DEPTH:
                        h = hh - DEPTH
                        ex = exs[h]
                        psX = psXp.tile([P, LT4, DH + 1], F32, tag="x4")
                        for lt in range(LT4):
                            nc.tensor.matmul(
                                psX[:, lt, :], ex[:, lt * P : (lt + 1) * P],
                                vpa[h][:],
                                start=True, stop=True,
                            )
                        rc = recpool.tile([P, LT4, 1], F32, tag="rc")
                        nc.vector.reciprocal(rc[:], psX[:, :, DH : DH + 1])
                        nc.vector.tensor_tensor(
                            ot[:, :, h * DH : (h + 1) * DH],
                            psX[:, :, 0:DH],
                            rc[:].to_broadcast([P, LT4, DH]),
                            mult,
                        )
                        j0 = h * DH
                        nc.sync.dma_start(
                            outr[:, lc * LT4 : (lc + 1) * LT4, j0 : j0 + DH],
                            ot[:, :, j0 : j0 + DH],
                        )

    return nc


def _get_program(has_bias):
    if has_bias not in _PROGRAM:
        _PROGRAM[has_bias] = _build_program(has_bias)
    return _PROGRAM[has_bias]


def kernel(x, Wq, bq, Wk, bk, Wv, bv, E):
    global LAST_RESULTS
    x = np.asarray(x, dtype=np.float32)
    Wq = np.asarray(Wq, dtype=np.float32)
    bq = np.asarray(bq, dtype=np.float32)
    Wk = np.asarray(Wk, dtype=np.float32)
    bk = np.asarray(bk, dtype=np.float32)
    Wv = np.asarray(Wv, dtype=np.float32)
    bv = np.asarray(bv, dtype=np.float32)
    E = np.asarray(E, dtype=np.float32)

    BF = ml_dtypes.bfloat16
    scale = 1.0 / math.sqrt(DH)
    # pass-A x: [lt, ll, d] — literally x[b] reshaped
    xLs = [np.ascontiguousarray(x[b].reshape(NLT, P, D).astype(BF)) for b in range(B)]
    # pass-B x: [d, l] -> [lc, pi, dc, lt, ll]
    xTs = [
        np.ascontiguousarray(
            x[b].T.reshape(DC, P, NLC, LT4, P).transpose(2, 1, 0, 3, 4).astype(BF)
        )
        for b in range(B)
    ]
    in_maps = []
    for core in range(NCORES):
        b = core % B
        hg = core // B
        js = slice(hg * J, (hg + 1) * J)
        hs = slice(hg * H, (hg + 1) * H)
        # W.T [D, J] -> [pi, dc, j] so per-partition lines are contiguous
        wqPs = np.ascontiguousarray(
            (Wq[js, :] * scale).T.reshape(DC, P, J).transpose(1, 0, 2).astype(BF)
        )
        wkPs = np.ascontiguousarray(
            Wk[js, :].T.reshape(DC, P, J).transpose(1, 0, 2).astype(BF)
        )
        wvPs = np.ascontiguousarray(
            Wv[js, :].T.reshape(DC, P, J).transpose(1, 0, 2).astype(BF)
        )
        bqTs = np.ascontiguousarray((bq[js] * scale).reshape(JT, P).T)
        E_s = E[hs]  # [H, KK, L]
        # [g, h4, kk, lt, ll] -> [lt, ll, g, h4*kk]
        eLs = np.ascontiguousarray(
            E_s.reshape(G4, 4, KK, NLT, P).transpose(3, 4, 0, 1, 2)
            .reshape(NLT, P, G4, KK4).astype(BF)
        )
        erowsTs = np.ascontiguousarray(E_s.sum(-1).reshape(1, H, KK).astype(BF))
        bkvs = np.ascontiguousarray(
            np.stack([bk[js].reshape(H, DH), bv[js].reshape(H, DH)])
            .reshape(1, 2, H, DH).astype(BF)
        )
        in_maps.append(
            {
                "xL": xLs[b],
                "xT": xTs[b],
                "wqP": wqPs,
                "wkP": wkPs,
                "wvP": wvPs,
                "bqT": bqTs,
                "eL": eLs,
                "erowsT": erowsTs,
                "bkv": bkvs,
            }
        )

    has_bias = bool(np.any(bk) or np.any(bv))
    nc = _get_program(has_bias)
    res = run_bass_kernel_spmd(nc, in_maps, list(range(NCORES)), trace=TRACE)
    LAST_RESULTS = res

    outp = np.empty((B, L, D), dtype=np.float32)
    for core in range(NCORES):
        b = core % B
        hg = core // B
        outp[b, :, hg * J : (hg + 1) * J] = res.results[core]["out"].astype(np.float32)
    return outp
